# revision 3
# baseline (speedup 1.0000x reference)
"""TRN2 Bass kernel for nn_Attention_20633022890922.

The reference module's einsum 'bqhk,bvhd->bqhd' contracts the attention-weight
head axis (k) and the value head axis (v) independently, so the product
factorizes into (sum_k softmax(...)) * (sum_v V) = 1 * Vsum.  The whole module
is therefore algebraically a single linear layer:

    out = tokens @ Wv_sum @ Wo_sum + bo
      Wv_sum[h, d]  = sum_v Wv[h, v*64 + d]          (512 x 64)
      Wo_sum[d, e]  = sum_q Wo[q*64 + d, e]          (64 x 512)

(The only approximation is softmax summing to 1.0, which holds to ~1e-7 in
fp32.)  Wq / Wk cancel entirely.

Device strategy: data-parallel over the batch dim (8 batches -> 8 cores).
Per core: Y = X @ Wv_sum @ Wo_sum with X [8192, 512].

The kernel is HBM-bound (358 GB/s per-core HBM cap), so every tensor crosses
HBM as fp16: X in (8 MB), Y out (8 MB), weights fp16 (error budget: measured
5.1e-4 max-rel end-to-end vs the 2e-2 gate; the PE quantizes matmul operands
to ~12 mantissa bits anyway, so fp16's 11-bit significand is nearly free).
The host casts X to fp16 AND pre-transposes it to [hid, token] layout so
every device DMA is a plain contiguous transfer; the host casts Y back up.

  GEMM1 (fp16, 4 accum matmuls / 512-token chunk), weight-stationary-outer
        so each stationary is reused across the wave's chunks:
        pt[0:64] = Wv_sum_j.T @ X_j^T  accumulated over 4 hid-blocks j
  GEMM2 (fp16, K=64, 1 matmul per 128-token tile):
        Y[t, :] = T @ Wo_sum
  bias bo is all-zero per the spec; if nonzero it is added on the host
  during unsharding.
"""

import time

import numpy as np

from concourse import bacc, mybir, tile
from concourse import bass_utils

B, N_TOK, HID, EMB, NH, HD = 8, 8192, 512, 512, 8, 64
N_CORES = 8
CH = 512                      # tokens per compute chunk
WAVE = 1024                   # tokens per load wave
NCHUNK = N_TOK // CH          # 16
NWAVE = N_TOK // WAVE         # 8
CPW = WAVE // CH              # chunks per wave = 2

F32 = mybir.dt.float32
FP16 = mybir.dt.float16

_compiled = None


def _build():
    nc = bacc.Bacc(
        trn_type="TRN2", target_bir_lowering=False, debug=False, num_devices=N_CORES
    )

    # host-transposed fp16 X: [4 hid-blocks, 128 hid, 8192 tokens]
    xf_d = nc.dram_tensor("xf", [4, 128, N_TOK], FP16, kind="ExternalInput")
    # packed consts, one DMA: [wv stationaries (4x64 cols) | wo (rows 0-63)]
    cw_d = nc.dram_tensor("cw", [128, 768], FP16, kind="ExternalInput")
    y_d = nc.dram_tensor("y", [N_TOK, HID], FP16, kind="ExternalOutput")

    with tile.TileContext(nc) as tc:
        with (
            tc.tile_pool(name="const", bufs=1) as constp,
            tc.tile_pool(name="xt", bufs=16) as xt_p,
            tc.tile_pool(name="tt", bufs=3) as tt_p,
            tc.tile_pool(name="yout", bufs=8) as y_p,
            tc.tile_pool(name="ps_t", bufs=4, space="PSUM") as ps_t,
            tc.tile_pool(name="ps_y", bufs=4, space="PSUM") as ps_y,
        ):
            cw = constp.tile([128, 768], FP16, tag="cw")
            # split const load: the first GEMM1 matmuls only need wv block 0
            nc.scalar.dma_start(cw[:, 0:256], cw_d[:, 0:256])
            nc.scalar.dma_start(cw[:, 256:768], cw_d[:, 256:768])
            wop = cw[0:64, 256:768]

            xt_by_wave = []
            for w in range(NWAVE):
                # ---- plain contiguous loads, one per hid-block (fine-grained
                # deps: the first GEMM1 matmuls only need block j=0)
                xt = []
                for j in range(4):
                    t = xt_p.tile([128, WAVE], FP16, tag="xt", name=f"xt{w}_{j}")
                    nc.sync.dma_start(t[:], xf_d[j, :, w * WAVE:(w + 1) * WAVE])
                    xt.append(t)
                xt_by_wave.append(xt)

            for w in range(NWAVE):
                xt = xt_by_wave[w]
                # ---- GEMM1: pt = T^T for the wave's chunks.  Wave 0 runs
                # chunk-major so chunk 0 finishes ASAP (the store stream is
                # the critical chain and its start time shifts everything);
                # later waves run weight-stationary-outer, which reuses each
                # stationary across the wave's chunks.
                pts = [ps_t.tile([64, CH], F32, tag="pt", name=f"pt{w}_{q}")
                       for q in range(CPW)]
                if w == 0:
                    for q in range(CPW):
                        for j in range(4):
                            ws = cw[:, j * 64:(j + 1) * 64]
                            nc.tensor.matmul(
                                pts[q][:], ws,
                                xt[j][:, q * CH:(q + 1) * CH],
                                start=(j == 0), stop=(j == 3),
                                skip_group_check=True,
                            )
                else:
                    for j in range(4):
                        ws = cw[:, j * 64:(j + 1) * 64]
                        for q in range(CPW):
                            nc.tensor.matmul(
                                pts[q][:], ws,
                                xt[j][:, q * CH:(q + 1) * CH],
                                start=(j == 0), stop=(j == 3),
                                skip_group_check=True,
                            )

                for q in range(CPW):
                    # ---- T^T to SBUF as fp16 for GEMM2
                    tt = tt_p.tile([64, CH], FP16, tag="tt")
                    nc.vector.tensor_copy(tt[:], pts[q][:])

                    # ---- GEMM2 (K=64): Y tile = T_tile @ Wo_sum
                    yo = y_p.tile([128, 4, HID], FP16, tag="yo")
                    for i in range(4):
                        py = ps_y.tile([128, HID], F32, tag="py")
                        nc.tensor.matmul(
                            py[:], tt[:, 128 * i:128 * (i + 1)], wop,
                            start=True, stop=True,
                        )
                        # PSUM->SBUF fp32->fp16 copies, balanced across the
                        # two engines with a PSUM port (gpsimd has none)
                        if i % 2 == 0:
                            nc.vector.tensor_copy(yo[:, i, :], py[:])
                        else:
                            nc.scalar.copy(yo[:, i, :], py[:])

                    c = w * CPW + q
                    ydst = y_d[c * CH:(c + 1) * CH, :].rearrange(
                        "(i p) h -> p i h", p=128
                    )
                    if c < NCHUNK - 1:
                        nc.scalar.dma_start(ydst, yo[:])
                    else:
                        # final chunk: 4 small stores on both rings so the
                        # last completion receipt is short and parallel
                        for i in range(4):
                            eng = nc.sync if i % 2 == 0 else nc.scalar
                            eng.dma_start(ydst[:, i, :], yo[:, i, :])

    nc.compile()
    return nc


def _get_compiled():
    global _compiled
    if _compiled is None:
        _compiled = _build()
    return _compiled


def kernel(tokens, Wq, Wk, Wv, Wo, bo, _trace=False):
    tokens = np.asarray(tokens, dtype=np.float32)
    Wv = np.asarray(Wv, dtype=np.float32)
    Wo = np.asarray(Wo, dtype=np.float32)
    bo = np.asarray(bo, dtype=np.float32)

    # Host-side prep: fold weights, cast everything to fp16, pre-transpose X
    # to hid-major so all device DMAs are contiguous.
    wv_sum = Wv.reshape(HID, NH, HD).sum(axis=1).astype(np.float32)
    wo_sum = Wo.reshape(NH, HD, HID).sum(axis=0).astype(np.float32)
    # GEMM1 stationaries: [k (hid within block), j*64 + m]
    wv_chip = (
        wv_sum.astype(np.float16).reshape(4, 128, HD).transpose(1, 0, 2)
        .reshape(128, 256)
    )
    cw = np.zeros((128, 768), dtype=np.float16)
    cw[:, 0:256] = wv_chip
    cw[0:64, 256:768] = wo_sum.astype(np.float16)

    xf = tokens.astype(np.float16)           # [B, N, 512]
    # -> [B, 4 hid-blocks, 128 hid, N tokens] (host-side transpose)
    xf = np.ascontiguousarray(xf.reshape(B, N_TOK, 4, 128).transpose(0, 2, 3, 1))

    nc = _get_compiled()
    in_maps = [
        {"xf": xf[b], "cw": cw}
        for b in range(N_CORES)
    ]
    # retry once or twice on transient device flakes (rare NRT_EXEC_UNIT
    # wedges have been observed under the axon PJRT path)
    for attempt in range(3):
        try:
            res = bass_utils.run_bass_kernel_spmd(
                nc, in_maps, core_ids=list(range(N_CORES)), trace=_trace
            )
            break
        except Exception:
            if attempt == 2:
                raise
            time.sleep(20)
    out = np.stack(
        [res.results[b]["y"].astype(np.float32) for b in range(N_CORES)], axis=0
    )
    if np.any(bo):
        out += bo
    if _trace:
        return out, res
    return out


if __name__ == "__main__":
    rng = np.random.default_rng(0)
    ins = {
        "tokens": rng.standard_normal((B, N_TOK, HID)).astype(np.float32),
        "Wq": (rng.standard_normal((HID, EMB)) * 0.02).astype(np.float32),
        "Wk": (rng.standard_normal((HID, EMB)) * 0.02).astype(np.float32),
        "Wv": (rng.standard_normal((HID, HID)) * 0.02).astype(np.float32),
        "Wo": (rng.standard_normal((EMB, HID)) * 0.02).astype(np.float32),
        "bo": np.zeros((HID,), dtype=np.float32),
    }
    out = kernel(**ins)
    print(out.shape, out.dtype)


# revision 6
# speedup vs baseline: 1.0945x; 1.0945x over previous
"""TRN2 Bass kernel for nn_Attention_20633022890922.

The reference module's einsum 'bqhk,bvhd->bqhd' contracts the attention-weight
head axis (k) and the value head axis (v) independently, so the product
factorizes into (sum_k softmax(...)) * (sum_v V) = 1 * Vsum.  The whole module
is therefore algebraically a single linear layer:

    out = tokens @ Wv_sum @ Wo_sum + bo
      Wv_sum[h, d]  = sum_v Wv[h, v*64 + d]          (512 x 64)
      Wo_sum[d, e]  = sum_q Wo[q*64 + d, e]          (64 x 512)

(The only approximation is softmax summing to 1.0, which holds to ~1e-7 in
fp32.)  Wq / Wk cancel entirely.

Device strategy: data-parallel over the batch dim (8 batches -> 8 cores).
Per core: Y = X @ Wv_sum @ Wo_sum with X [8192, 512].

Everything crosses HBM as fp16 (X in: 8 MB, Y out: 8 MB, tiny weights); the
host pre-transposes X to hid-major so all device DMAs are contiguous, and
casts Y back up.  Error budget: 5.1e-4 max-rel end-to-end vs the 2e-2 gate
(the PE quantizes matmul operands to ~12 mantissa bits anyway).

The kernel is jointly PE- and HBM-paced (~27us of matmul streaming vs ~46us
of DMA at the 358 GB/s per-core cap), so the PE stream must stay dense and
warm (HAM throttles an idle PE to 1.2 GHz):
  - warmup matmuls on garbage keep the PE at 2.4 GHz until real data lands;
  - all 8 X waves are resident in SBUF (no flow-control starvation);
  - GEMM1 of wave w+1 is emitted before GEMM2 of wave w so the PSUM->SBUF
    cast latency hides under matmul streaming;
  - GEMM1 duplicates the T rows (stationary [128h, 64+64 dup] - free, the
    PE streams by moving-operand columns) so GEMM2 can alternate its
    stationary between array row-halves via tile_position; the LDWEIGHTS
    of one half overlaps the in-flight matmul on the other half.
"""

import time

import numpy as np

from concourse import bacc, mybir, tile
from concourse import bass_utils

B, N_TOK, HID, EMB, NH, HD = 8, 8192, 512, 512, 8, 64
N_CORES = 8
CH = 512                      # tokens per compute chunk
WAVE = 1024                   # tokens per load wave
NCHUNK = N_TOK // CH          # 16
NWAVE = N_TOK // WAVE         # 8
CPW = WAVE // CH              # chunks per wave = 2

F32 = mybir.dt.float32
FP16 = mybir.dt.float16

_compiled = None


def _build():
    nc = bacc.Bacc(
        trn_type="TRN2", target_bir_lowering=False, debug=False, num_devices=N_CORES
    )

    # host-transposed fp16 X: [4 hid-blocks, 128 hid, 8192 tokens]
    xf_d = nc.dram_tensor("xf", [4, 128, N_TOK], FP16, kind="ExternalInput")
    # packed consts: [4 x (wv_j | wv_j) stationaries, 512 cols | wo-dup 512]
    cw_d = nc.dram_tensor("cw", [128, 1024], FP16, kind="ExternalInput")
    y_d = nc.dram_tensor("y", [N_TOK, HID], FP16, kind="ExternalOutput")

    with tile.TileContext(nc) as tc:
        with (
            tc.tile_pool(name="const", bufs=1) as constp,
            tc.tile_pool(name="warm", bufs=1) as warm_p,
            tc.tile_pool(name="xt", bufs=32) as xt_p,
            tc.tile_pool(name="tt", bufs=3) as tt_p,
            tc.tile_pool(name="yout", bufs=8) as y_p,
            tc.tile_pool(name="ps_t", bufs=4, space="PSUM") as ps_t,
            tc.tile_pool(name="ps_y", bufs=4, space="PSUM") as ps_y,
        ):
            cw = constp.tile([128, 1024], FP16, tag="cw")
            # split const load: the first GEMM1 matmuls only need wv block 0
            nc.scalar.dma_start(cw[:, 0:512], cw_d[:, 0:512])
            nc.scalar.dma_start(cw[:, 512:1024], cw_d[:, 512:1024])

            # ---- PE warmup: HAM throttles the PE to 1.2 GHz until it has
            # been busy ~3.4us, and re-throttles after ~3.4us idle.  Real
            # matmuls can't start until the first X wave lands (~12us in:
            # runtime preamble + first DMA bytes).  Keep the PE busy on
            # garbage from t=0 so the real stream runs at 2.4 GHz.
            wsrc = warm_p.tile([128, CH], FP16, tag="wsrc")
            nc.vector.memset(wsrc[:], 1.0)
            wps = [ps_y.tile([128, HID], F32, tag="py", name=f"wps{i}")
                   for i in range(2)]
            for k in range(40):
                nc.tensor.matmul(
                    wps[k % 2][:], wsrc[:, 0:128], wsrc[:],
                    start=True, stop=True, skip_group_check=True,
                )

            # ---- all waves resident: loads stream at line rate, the PE
            # never waits on flow control
            xt_by_wave = []
            for w in range(NWAVE):
                xt = []
                for j in range(4):
                    t = xt_p.tile([128, WAVE], FP16, tag="xt", name=f"xt{w}_{j}")
                    nc.sync.dma_start(t[:], xf_d[j, :, w * WAVE:(w + 1) * WAVE])
                    xt.append(t)
                xt_by_wave.append(xt)

            def gemm1(w):
                xt = xt_by_wave[w]
                pts = [ps_t.tile([128, CH], F32, tag="pt", name=f"pt{w}_{q}")
                       for q in range(CPW)]
                for j in range(4):
                    ws = cw[:, j * 128:(j + 1) * 128]
                    for q in range(CPW):
                        nc.tensor.matmul(
                            pts[q][:], ws,
                            xt[j][:, q * CH:(q + 1) * CH],
                            start=(j == 0), stop=(j == 3),
                            skip_group_check=True,
                        )
                return pts

            def gemm2(w, pts):
                for q in range(CPW):
                    c = w * CPW + q
                    # T^T (rows duplicated) to SBUF as fp16 for GEMM2;
                    # alternate the cast between the two PSUM-capable
                    # engines to balance their load
                    tt = tt_p.tile([128, CH], FP16, tag="tt")
                    if c % 2 == 0:
                        nc.vector.tensor_copy(tt[:], pts[q][:])
                    else:
                        nc.scalar.copy(tt[:], pts[q][:])

                    yo = y_p.tile([128, 4, HID], FP16, tag="yo")
                    for i in range(4):
                        py = ps_y.tile([128, HID], F32, tag="py")
                        # GEMM2 (K=64): alternate the stationary between
                        # array row-halves so each LDWEIGHTS overlaps the
                        # in-flight matmul on the other half
                        h = (i % 2) * 64
                        nc.tensor.matmul(
                            py[:], tt[h:h + 64, 128 * i:128 * (i + 1)],
                            cw[h:h + 64, 512:1024],
                            start=True, stop=True,
                            tile_position=(h, 0),
                        )
                        # PSUM->SBUF fp32->fp16 copies, balanced across the
                        # two engines with a PSUM port (gpsimd has none)
                        if i % 2 == 0:
                            nc.vector.tensor_copy(yo[:, i, :], py[:])
                        else:
                            nc.scalar.copy(yo[:, i, :], py[:])

                    ydst = y_d[c * CH:(c + 1) * CH, :].rearrange(
                        "(i p) h -> p i h", p=128
                    )
                    if c < NCHUNK - 1:
                        nc.scalar.dma_start(ydst, yo[:])
                    else:
                        # final chunk: 4 small stores on both rings so the
                        # last completion receipt is short and parallel
                        for i in range(4):
                            eng = nc.sync if i % 2 == 0 else nc.scalar
                            eng.dma_start(ydst[:, i, :], yo[:, i, :])

            # ---- software-pipelined waves: GEMM1(w+1) is emitted (and so
            # runs on the PE) before GEMM2(w), hiding the cast latency
            pts_prev = gemm1(0)
            for w in range(1, NWAVE):
                pts_cur = gemm1(w)
                gemm2(w - 1, pts_prev)
                pts_prev = pts_cur
            gemm2(NWAVE - 1, pts_prev)

    nc.compile()
    return nc


def _get_compiled():
    global _compiled
    if _compiled is None:
        _compiled = _build()
    return _compiled


def kernel(tokens, Wq, Wk, Wv, Wo, bo, _trace=False):
    tokens = np.asarray(tokens, dtype=np.float32)
    Wv = np.asarray(Wv, dtype=np.float32)
    Wo = np.asarray(Wo, dtype=np.float32)
    bo = np.asarray(bo, dtype=np.float32)

    # Host-side prep: fold weights, cast everything to fp16, pre-transpose X
    # to hid-major so all device DMAs are contiguous.
    wv_sum = Wv.reshape(HID, NH, HD).sum(axis=1).astype(np.float32)
    wo_sum = Wo.reshape(NH, HD, HID).sum(axis=0).astype(np.float32)
    wv16 = wv_sum.astype(np.float16)
    wo16 = wo_sum.astype(np.float16)
    cw = np.zeros((128, 1024), dtype=np.float16)
    for j in range(4):
        blk = wv16[j * 128:(j + 1) * 128, :]          # [128, 64]
        cw[:, j * 128:j * 128 + 64] = blk
        cw[:, j * 128 + 64:(j + 1) * 128] = blk       # duplicated T rows
    cw[0:64, 512:1024] = wo16
    cw[64:128, 512:1024] = wo16                       # wo on both row-halves

    xf = tokens.astype(np.float16)           # [B, N, 512]
    # -> [B, 4 hid-blocks, 128 hid, N tokens] (host-side transpose)
    xf = np.ascontiguousarray(xf.reshape(B, N_TOK, 4, 128).transpose(0, 2, 3, 1))

    nc = _get_compiled()
    in_maps = [
        {"xf": xf[b], "cw": cw}
        for b in range(N_CORES)
    ]
    # retry once or twice on transient device flakes (rare NRT_EXEC_UNIT
    # wedges have been observed under the axon PJRT path)
    for attempt in range(3):
        try:
            res = bass_utils.run_bass_kernel_spmd(
                nc, in_maps, core_ids=list(range(N_CORES)), trace=_trace
            )
            break
        except Exception:
            if attempt == 2:
                raise
            time.sleep(20)
    out = np.stack(
        [res.results[b]["y"].astype(np.float32) for b in range(N_CORES)], axis=0
    )
    if np.any(bo):
        out += bo
    if _trace:
        return out, res
    return out


if __name__ == "__main__":
    rng = np.random.default_rng(0)
    ins = {
        "tokens": rng.standard_normal((B, N_TOK, HID)).astype(np.float32),
        "Wq": (rng.standard_normal((HID, EMB)) * 0.02).astype(np.float32),
        "Wk": (rng.standard_normal((HID, EMB)) * 0.02).astype(np.float32),
        "Wv": (rng.standard_normal((HID, HID)) * 0.02).astype(np.float32),
        "Wo": (rng.standard_normal((EMB, HID)) * 0.02).astype(np.float32),
        "bo": np.zeros((HID,), dtype=np.float32),
    }
    out = kernel(**ins)
    print(out.shape, out.dtype)


# revision 8
# speedup vs baseline: 1.1525x; 1.0530x over previous
"""TRN2 Bass kernel for nn_Attention_20633022890922.

The reference module's einsum 'bqhk,bvhd->bqhd' contracts the attention-weight
head axis (k) and the value head axis (v) independently, so the product
factorizes into (sum_k softmax(...)) * (sum_v V) = 1 * Vsum.  The whole module
is therefore algebraically a single linear layer:

    out = tokens @ Wv_sum @ Wo_sum + bo
      Wv_sum[h, d]  = sum_v Wv[h, v*64 + d]          (512 x 64)
      Wo_sum[d, e]  = sum_q Wo[q*64 + d, e]          (64 x 512)

(The only approximation is softmax summing to 1.0, which holds to ~1e-7 in
fp32.)  Wq / Wk cancel entirely.

Device strategy: data-parallel over the batch dim (8 batches -> 8 cores).
Per core: Y = X @ Wv_sum @ Wo_sum with X [8192, 512].

Everything crosses HBM as fp16 (X in: 8 MB, Y out: 8 MB, tiny weights); the
host pre-transposes X to hid-major so all device DMAs are contiguous, and
casts Y back up.  Error budget: 5.1e-4 max-rel end-to-end vs the 2e-2 gate
(the PE quantizes matmul operands to ~12 mantissa bits anyway).

The kernel is jointly PE- and HBM-paced (~27us of matmul streaming vs ~46us
of DMA at the 358 GB/s per-core cap), so the PE stream must stay dense and
warm (HAM throttles an idle PE to 1.2 GHz):
  - warmup matmuls on garbage keep the PE at 2.4 GHz until real data lands;
  - all 8 X waves are resident in SBUF (no flow-control starvation);
  - GEMM1 of wave w+1 is emitted before GEMM2 of wave w so the PSUM->SBUF
    cast latency hides under matmul streaming;
  - GEMM1 duplicates the T rows (stationary [128h, 64+64 dup] - free, the
    PE streams by moving-operand columns) so GEMM2 can alternate its
    stationary between array row-halves via tile_position; the LDWEIGHTS
    of one half overlaps the in-flight matmul on the other half.
"""

import time

import numpy as np

from concourse import bacc, mybir, tile
from concourse import bass_utils

B, N_TOK, HID, EMB, NH, HD = 8, 8192, 512, 512, 8, 64
N_CORES = 8
CH = 512                      # tokens per compute chunk
WAVE = 1024                   # tokens per load wave
NCHUNK = N_TOK // CH          # 16
NWAVE = N_TOK // WAVE         # 8
CPW = WAVE // CH              # chunks per wave = 2

F32 = mybir.dt.float32
FP16 = mybir.dt.float16

_compiled = None


def _build():
    nc = bacc.Bacc(
        trn_type="TRN2", target_bir_lowering=False, debug=False, num_devices=N_CORES
    )

    # host-transposed fp16 X: [4 hid-blocks, 128 hid, 8192 tokens]
    xf_d = nc.dram_tensor("xf", [4, 128, N_TOK], FP16, kind="ExternalInput")
    # packed consts: [4 x (wv_j | wv_j) stationaries, 512 cols | wo-dup 512]
    cw_d = nc.dram_tensor("cw", [128, 1024], FP16, kind="ExternalInput")
    y_d = nc.dram_tensor("y", [N_TOK, HID], FP16, kind="ExternalOutput")

    with tile.TileContext(nc) as tc:
        with (
            tc.tile_pool(name="const", bufs=1) as constp,
            tc.tile_pool(name="xt", bufs=8) as xt_p,
            tc.tile_pool(name="tt", bufs=3) as tt_p,
            tc.tile_pool(name="yout", bufs=4) as y_p,
            tc.tile_pool(name="ps_t", bufs=4, space="PSUM") as ps_t,
            tc.tile_pool(name="ps_y", bufs=4, space="PSUM") as ps_y,
        ):
            cw = constp.tile([128, 1024], FP16, tag="cw")
            # split const load: the first GEMM1 matmuls only need wv block 0
            nc.scalar.dma_start(cw[:, 0:512], cw_d[:, 0:512])
            nc.scalar.dma_start(cw[:, 512:1024], cw_d[:, 512:1024])

            # ---- all waves resident: loads stream at line rate, the PE
            # never waits on flow control.  Wave 0 loads per hid-block so
            # the first GEMM1 matmul starts as early as possible; later
            # waves load 1 MB at a time (fewer, cheaper issue ops).
            xt_by_wave = []
            for w in range(NWAVE):
                t = xt_p.tile([128, 4 * WAVE], FP16, tag="xt", name=f"xt{w}")
                src = xf_d[:, :, w * WAVE:(w + 1) * WAVE]
                for j in range(4):
                    nc.sync.dma_start(t[:, j * WAVE:(j + 1) * WAVE], src[j])
                xt_by_wave.append(t)

            def gemm1(w):
                xt = xt_by_wave[w]
                pts = [ps_t.tile([128, CH], F32, tag="pt", name=f"pt{w}_{q}")
                       for q in range(CPW)]
                for j in range(4):
                    ws = cw[:, j * 128:(j + 1) * 128]
                    for q in range(CPW):
                        nc.tensor.matmul(
                            pts[q][:], ws,
                            xt[:, j * WAVE + q * CH:j * WAVE + (q + 1) * CH],
                            start=(j == 0), stop=(j == 3),
                            skip_group_check=True,
                        )
                return pts

            def gemm2(w, pts):
                yo = y_p.tile([128, 2 * 4, HID], FP16, tag="yo")
                for q in range(CPW):
                    c = w * CPW + q
                    # T^T (rows duplicated) to SBUF as fp16 for GEMM2;
                    # alternate the cast between the two PSUM-capable
                    # engines to balance their load
                    tt = tt_p.tile([128, CH], FP16, tag="tt")
                    if c % 2 == 0:
                        nc.vector.tensor_copy(tt[:], pts[q][:])
                    else:
                        nc.scalar.copy(tt[:], pts[q][:])

                    for i in range(4):
                        py = ps_y.tile([128, HID], F32, tag="py")
                        # GEMM2 (K=64): alternate the stationary between
                        # array row-halves so each LDWEIGHTS overlaps the
                        # in-flight matmul on the other half
                        h = (i % 2) * 64
                        nc.tensor.matmul(
                            py[:], tt[h:h + 64, 128 * i:128 * (i + 1)],
                            cw[h:h + 64, 512:1024],
                            start=True, stop=True,
                            tile_position=(h, 0),
                        )
                        # PSUM->SBUF fp32->fp16 copies, balanced across the
                        # two engines with a PSUM port (gpsimd has none)
                        if i % 2 == 0:
                            nc.vector.tensor_copy(yo[:, q * 4 + i, :], py[:])
                        else:
                            nc.scalar.copy(yo[:, q * 4 + i, :], py[:])

                ydst = y_d[w * WAVE:(w + 1) * WAVE, :].rearrange(
                    "(i p) h -> p i h", p=128
                )
                if w < NWAVE - 1:
                    nc.scalar.dma_start(ydst, yo[:])
                else:
                    # final wave: 4 small stores on both rings so the
                    # last completion receipt is short and parallel
                    for i in range(4):
                        eng = nc.sync if i % 2 == 0 else nc.scalar
                        eng.dma_start(
                            ydst[:, 2 * i:2 * (i + 1), :],
                            yo[:, 2 * i:2 * (i + 1), :],
                        )

            # ---- software-pipelined waves: GEMM1(w+1) is emitted (and so
            # runs on the PE) before GEMM2(w), hiding the cast latency
            pts_prev = gemm1(0)
            for w in range(1, NWAVE):
                pts_cur = gemm1(w)
                gemm2(w - 1, pts_prev)
                pts_prev = pts_cur
            gemm2(NWAVE - 1, pts_prev)

    nc.compile()
    return nc


def _get_compiled():
    global _compiled
    if _compiled is None:
        _compiled = _build()
    return _compiled


def kernel(tokens, Wq, Wk, Wv, Wo, bo, _trace=False):
    tokens = np.asarray(tokens, dtype=np.float32)
    Wv = np.asarray(Wv, dtype=np.float32)
    Wo = np.asarray(Wo, dtype=np.float32)
    bo = np.asarray(bo, dtype=np.float32)

    # Host-side prep: fold weights, cast everything to fp16, pre-transpose X
    # to hid-major so all device DMAs are contiguous.
    wv_sum = Wv.reshape(HID, NH, HD).sum(axis=1).astype(np.float32)
    wo_sum = Wo.reshape(NH, HD, HID).sum(axis=0).astype(np.float32)
    wv16 = wv_sum.astype(np.float16)
    wo16 = wo_sum.astype(np.float16)
    cw = np.zeros((128, 1024), dtype=np.float16)
    for j in range(4):
        blk = wv16[j * 128:(j + 1) * 128, :]          # [128, 64]
        cw[:, j * 128:j * 128 + 64] = blk
        cw[:, j * 128 + 64:(j + 1) * 128] = blk       # duplicated T rows
    cw[0:64, 512:1024] = wo16
    cw[64:128, 512:1024] = wo16                       # wo on both row-halves

    xf = tokens.astype(np.float16)           # [B, N, 512]
    # -> [B, 4 hid-blocks, 128 hid, N tokens] (host-side transpose)
    xf = np.ascontiguousarray(xf.reshape(B, N_TOK, 4, 128).transpose(0, 2, 3, 1))

    nc = _get_compiled()
    in_maps = [
        {"xf": xf[b], "cw": cw}
        for b in range(N_CORES)
    ]
    # retry once or twice on transient device flakes (rare NRT_EXEC_UNIT
    # wedges have been observed under the axon PJRT path)
    for attempt in range(3):
        try:
            res = bass_utils.run_bass_kernel_spmd(
                nc, in_maps, core_ids=list(range(N_CORES)), trace=_trace
            )
            break
        except Exception:
            if attempt == 2:
                raise
            time.sleep(20)
    out = np.stack(
        [res.results[b]["y"].astype(np.float32) for b in range(N_CORES)], axis=0
    )
    if np.any(bo):
        out += bo
    if _trace:
        return out, res
    return out


if __name__ == "__main__":
    rng = np.random.default_rng(0)
    ins = {
        "tokens": rng.standard_normal((B, N_TOK, HID)).astype(np.float32),
        "Wq": (rng.standard_normal((HID, EMB)) * 0.02).astype(np.float32),
        "Wk": (rng.standard_normal((HID, EMB)) * 0.02).astype(np.float32),
        "Wv": (rng.standard_normal((HID, HID)) * 0.02).astype(np.float32),
        "Wo": (rng.standard_normal((EMB, HID)) * 0.02).astype(np.float32),
        "bo": np.zeros((HID,), dtype=np.float32),
    }
    out = kernel(**ins)
    print(out.shape, out.dtype)


# revision 10
# speedup vs baseline: 1.1528x; 1.0003x over previous
"""TRN2 Bass kernel for nn_Attention_20633022890922.

The reference module's einsum 'bqhk,bvhd->bqhd' contracts the attention-weight
head axis (k) and the value head axis (v) independently, so the product
factorizes into (sum_k softmax(...)) * (sum_v V) = 1 * Vsum.  The whole module
is therefore algebraically a single linear layer:

    out = tokens @ Wv_sum @ Wo_sum + bo
      Wv_sum[h, d]  = sum_v Wv[h, v*64 + d]          (512 x 64)
      Wo_sum[d, e]  = sum_q Wo[q*64 + d, e]          (64 x 512)

(The only approximation is softmax summing to 1.0, which holds to ~1e-7 in
fp32.)  Wq / Wk cancel entirely.

Device strategy: data-parallel over the batch dim (8 batches -> 8 cores).
Per core: Y = X @ Wv_sum @ Wo_sum with X [8192, 512].

Everything crosses HBM as fp16 (X in: 8 MB, Y out: 8 MB, tiny weights); the
host pre-transposes X to hid-major so all device DMAs are contiguous, and
casts Y back up.  Error budget: 5.1e-4 max-rel end-to-end vs the 2e-2 gate
(the PE quantizes matmul operands to ~12 mantissa bits anyway).

The kernel is jointly PE- and HBM-paced (~27us of matmul streaming vs ~46us
of DMA at the 358 GB/s per-core cap), so the PE stream must stay dense and
warm (HAM throttles an idle PE to 1.2 GHz):
  - warmup matmuls on garbage keep the PE at 2.4 GHz until real data lands;
  - all 8 X waves are resident in SBUF (no flow-control starvation);
  - GEMM1 of wave w+1 is emitted before GEMM2 of wave w so the PSUM->SBUF
    cast latency hides under matmul streaming;
  - GEMM1 duplicates the T rows (stationary [128h, 64+64 dup] - free, the
    PE streams by moving-operand columns) so GEMM2 can alternate its
    stationary between array row-halves via tile_position; the LDWEIGHTS
    of one half overlaps the in-flight matmul on the other half.
"""

import time

import numpy as np

from concourse import bacc, mybir, tile
from concourse import bass_utils

B, N_TOK, HID, EMB, NH, HD = 8, 8192, 512, 512, 8, 64
N_CORES = 8
CH = 512                      # tokens per compute chunk
WAVE = 1024                   # tokens per load wave
NCHUNK = N_TOK // CH          # 16
NWAVE = N_TOK // WAVE         # 8
CPW = WAVE // CH              # chunks per wave = 2

F32 = mybir.dt.float32
FP16 = mybir.dt.float16

_compiled = None


def _build():
    nc = bacc.Bacc(
        trn_type="TRN2", target_bir_lowering=False, debug=False, num_devices=N_CORES
    )

    # host-transposed fp16 X: [4 hid-blocks, 128 hid, 8192 tokens]
    xf_d = nc.dram_tensor("xf", [4, 128, N_TOK], FP16, kind="ExternalInput")
    # packed consts: [4 x (wv_j | wv_j) stationaries, 512 cols | wo-dup 512]
    cw_d = nc.dram_tensor("cw", [128, 1024], FP16, kind="ExternalInput")
    y_d = nc.dram_tensor("y", [N_TOK, HID], FP16, kind="ExternalOutput")

    with tile.TileContext(nc) as tc:
        with (
            tc.tile_pool(name="const", bufs=1) as constp,
            tc.tile_pool(name="xt", bufs=8) as xt_p,
            tc.tile_pool(name="tt", bufs=3) as tt_p,
            tc.tile_pool(name="yout", bufs=4) as y_p,
            tc.tile_pool(name="ps_t", bufs=4, space="PSUM") as ps_t,
            tc.tile_pool(name="ps_y", bufs=4, space="PSUM") as ps_y,
        ):
            cw = constp.tile([128, 1024], FP16, tag="cw")
            # split const load: the GEMM1 stationaries go first on the sync
            # ring (shortest path to the first LDWEIGHTS); the GEMM2 weights
            # aren't needed until ~8us later
            nc.sync.dma_start(cw[:, 0:512], cw_d[:, 0:512])
            nc.scalar.dma_start(cw[:, 512:1024], cw_d[:, 512:1024])

            # ---- all waves resident: loads stream at line rate, the PE
            # never waits on flow control.  Wave 0 loads per hid-block so
            # the first GEMM1 matmul starts as early as possible; later
            # waves load 1 MB at a time (fewer, cheaper issue ops).
            xt_by_wave = []
            for w in range(NWAVE):
                t = xt_p.tile([128, 4 * WAVE], FP16, tag="xt", name=f"xt{w}")
                src = xf_d[:, :, w * WAVE:(w + 1) * WAVE]
                for j in range(4):
                    nc.sync.dma_start(t[:, j * WAVE:(j + 1) * WAVE], src[j])
                xt_by_wave.append(t)

            def gemm1(w):
                xt = xt_by_wave[w]
                pts = [ps_t.tile([128, CH], F32, tag="pt", name=f"pt{w}_{q}")
                       for q in range(CPW)]
                for j in range(4):
                    ws = cw[:, j * 128:(j + 1) * 128]
                    for q in range(CPW):
                        nc.tensor.matmul(
                            pts[q][:], ws,
                            xt[:, j * WAVE + q * CH:j * WAVE + (q + 1) * CH],
                            start=(j == 0), stop=(j == 3),
                            skip_group_check=True,
                        )
                return pts

            def gemm2(w, pts):
                yo = y_p.tile([128, 2 * 4, HID], FP16, tag="yo")
                for q in range(CPW):
                    c = w * CPW + q
                    # T^T (rows duplicated) to SBUF as fp16 for GEMM2;
                    # alternate the cast between the two PSUM-capable
                    # engines to balance their load
                    tt = tt_p.tile([128, CH], FP16, tag="tt")
                    if c % 2 == 0:
                        nc.vector.tensor_copy(tt[:], pts[q][:])
                    else:
                        nc.scalar.copy(tt[:], pts[q][:])

                    for i in range(4):
                        py = ps_y.tile([128, HID], F32, tag="py")
                        # GEMM2 (K=64): alternate the stationary between
                        # array row-halves so each LDWEIGHTS overlaps the
                        # in-flight matmul on the other half
                        h = (i % 2) * 64
                        nc.tensor.matmul(
                            py[:], tt[h:h + 64, 128 * i:128 * (i + 1)],
                            cw[h:h + 64, 512:1024],
                            start=True, stop=True,
                            tile_position=(h, 0),
                        )
                        # PSUM->SBUF fp32->fp16 copies, balanced across the
                        # two engines with a PSUM port (gpsimd has none)
                        if i % 2 == 0:
                            nc.vector.tensor_copy(yo[:, q * 4 + i, :], py[:])
                        else:
                            nc.scalar.copy(yo[:, q * 4 + i, :], py[:])

                ydst = y_d[w * WAVE:(w + 1) * WAVE, :].rearrange(
                    "(i p) h -> p i h", p=128
                )
                if w < NWAVE - 1:
                    nc.scalar.dma_start(ydst, yo[:])
                else:
                    # final wave: 4 small stores on both rings so the
                    # last completion receipt is short and parallel
                    for i in range(4):
                        eng = nc.sync if i % 2 == 0 else nc.scalar
                        eng.dma_start(
                            ydst[:, 2 * i:2 * (i + 1), :],
                            yo[:, 2 * i:2 * (i + 1), :],
                        )

            # ---- software-pipelined waves: GEMM1(w+1) is emitted (and so
            # runs on the PE) before GEMM2(w), hiding the cast latency
            pts_prev = gemm1(0)
            for w in range(1, NWAVE):
                pts_cur = gemm1(w)
                gemm2(w - 1, pts_prev)
                # ---- HAM fillers: the kernel is DMA-paced, so the PE idles
                # ~2.5us per wave waiting for the next X wave; repeated idle
                # windows re-throttle the PE to 1.2 GHz (measured 27us at
                # half clock).  Garbage matmuls into the just-consumed PSUM
                # tile keep the clock warm; the next real matmul overwrites
                # the bank (start=True).
                for f in range(6):
                    nc.tensor.matmul(
                        pts_prev[0][:], cw[:, 0:128], cw[:, 0:512],
                        start=True, stop=True, skip_group_check=True,
                    )
                pts_prev = pts_cur
            gemm2(NWAVE - 1, pts_prev)

    nc.compile()
    return nc


def _get_compiled():
    global _compiled
    if _compiled is None:
        _compiled = _build()
    return _compiled


def kernel(tokens, Wq, Wk, Wv, Wo, bo, _trace=False):
    tokens = np.asarray(tokens, dtype=np.float32)
    Wv = np.asarray(Wv, dtype=np.float32)
    Wo = np.asarray(Wo, dtype=np.float32)
    bo = np.asarray(bo, dtype=np.float32)

    # Host-side prep: fold weights, cast everything to fp16, pre-transpose X
    # to hid-major so all device DMAs are contiguous.
    wv_sum = Wv.reshape(HID, NH, HD).sum(axis=1).astype(np.float32)
    wo_sum = Wo.reshape(NH, HD, HID).sum(axis=0).astype(np.float32)
    wv16 = wv_sum.astype(np.float16)
    wo16 = wo_sum.astype(np.float16)
    cw = np.zeros((128, 1024), dtype=np.float16)
    for j in range(4):
        blk = wv16[j * 128:(j + 1) * 128, :]          # [128, 64]
        cw[:, j * 128:j * 128 + 64] = blk
        cw[:, j * 128 + 64:(j + 1) * 128] = blk       # duplicated T rows
    cw[0:64, 512:1024] = wo16
    cw[64:128, 512:1024] = wo16                       # wo on both row-halves

    xf = tokens.astype(np.float16)           # [B, N, 512]
    # -> [B, 4 hid-blocks, 128 hid, N tokens] (host-side transpose)
    xf = np.ascontiguousarray(xf.reshape(B, N_TOK, 4, 128).transpose(0, 2, 3, 1))

    nc = _get_compiled()
    in_maps = [
        {"xf": xf[b], "cw": cw}
        for b in range(N_CORES)
    ]
    # retry once or twice on transient device flakes (rare NRT_EXEC_UNIT
    # wedges have been observed under the axon PJRT path)
    for attempt in range(3):
        try:
            res = bass_utils.run_bass_kernel_spmd(
                nc, in_maps, core_ids=list(range(N_CORES)), trace=_trace
            )
            break
        except Exception:
            if attempt == 2:
                raise
            time.sleep(20)
    out = np.stack(
        [res.results[b]["y"].astype(np.float32) for b in range(N_CORES)], axis=0
    )
    if np.any(bo):
        out += bo
    if _trace:
        return out, res
    return out


if __name__ == "__main__":
    rng = np.random.default_rng(0)
    ins = {
        "tokens": rng.standard_normal((B, N_TOK, HID)).astype(np.float32),
        "Wq": (rng.standard_normal((HID, EMB)) * 0.02).astype(np.float32),
        "Wk": (rng.standard_normal((HID, EMB)) * 0.02).astype(np.float32),
        "Wv": (rng.standard_normal((HID, HID)) * 0.02).astype(np.float32),
        "Wo": (rng.standard_normal((EMB, HID)) * 0.02).astype(np.float32),
        "bo": np.zeros((HID,), dtype=np.float32),
    }
    out = kernel(**ins)
    print(out.shape, out.dtype)
